# revision 1
# baseline (speedup 1.0000x reference)
"""Trainium2 Bass kernel for nn_CinST_weight_59304908423199.

Computes the reference spatio-temporal gating module: per clip (8 segments),
channel avg+max pool -> 3x3x3 conv -> sigmoid channel gate, then temporal /
height / width avg+max pools of the gated features -> 3x3x3 convs -> sigmoid
gates, output = mean of the three broadcast gates.

Sharding: data-parallel over the 8 clips (batch dim), one clip per NeuronCore.

Implementation notes:
  - x cast fp32->bf16 during the load DMA (SWDGE); per-(t, c-group) tiles
    [128c, 3200] (flat h*56+w padded to 25*128 for the xbar transpose).
  - channel-max via bf16 xbar chunked transpose + DVE tree over free dim;
    channel-sum via PE "staircase" matmuls (sliding ones-column lhsT).
  - the four 3x3x3 convs run on PE as banded matmuls: the contraction dim
    carries a tridiagonal band (host-precomputed lhsT, mean-pool scale folded
    in); remaining two stencil axes are free-dim AP shifts on zero-padded
    grids. The c=127/128 group boundary of the c-banded convs is fixed with
    small DVE stencil corrections injected via K=1 one-hot matmuls.
  - pools: DVE bf16 running/tree reductions; gates+BN+sigmoid on ACT;
    final out = (yt+yh+yw)/3 via broadcast adds, cast bf16->fp32 on store.
"""
import numpy as np
import ml_dtypes

bf16 = ml_dtypes.bfloat16

T, C, H, W = 8, 256, 56, 56
FLAT = H * W            # 3136
FLATP = 3200            # 25 * 128
NCH = 25
EPS = 1e-5
NCORES = 8


def _patch_tile_drain():
    """This walrus build only accepts one sync-wait per CTRL (Drain) instruction:
    spread the TileContext final-drain waits across multiple drains."""
    import concourse.tile as tile
    from concourse.vector_clock import ScopedClock

    if getattr(tile.TileContext, "_drain_patched", False):
        return

    def _drain_and_barrier(self, tick_clock, wait_clock):
        nc = self.nc
        drain_inst = nc.sync.drain()
        wait_clock.add_sem_waits(
            drain_inst.ins, ScopedClock({None: tick_clock.global_clock})
        )
        si = drain_inst.ins.sync_info
        if si is not None and si.on_wait and len(si.on_wait) > 1:
            waits = list(si.on_wait)
            si.on_wait = waits[:1]
            for wv in waits[1:]:
                d2 = nc.sync.drain()
                si2 = d2.ins.sync_info
                if si2 is None:
                    from concourse import mybir
                    d2.ins.sync_info = mybir.SyncInfo(on_wait=[wv], on_update=[])
                else:
                    si2.on_wait = [wv]
        nc.all_engine_barrier()
        assert self.sems is not None
        popped = nc._tile_sem_poison_stack.pop()
        assert popped is self._sem_poison
        nc.clear_and_free_semaphores(list(self.sems.allocated().values()))
        nc.all_engine_barrier()

    tile.TileContext._drain_and_barrier = _drain_and_barrier
    tile.TileContext._drain_patched = True


def make_consts(Wc, Wt, Wh, Ww):
    def band(vals, n):
        b = np.zeros((n, n), np.float32)
        for d, v in ((-1, vals[0]), (0, vals[1]), (1, vals[2])):
            idx = np.arange(max(0, -d), min(n, n - d))
            b[idx + d, idx] = v
        return b

    def bands18(Wm, axis, n, msc):
        out = np.zeros((2, 3, 3, n, n), np.float32)
        for ch in range(2):
            s = msc if ch == 1 else 1.0
            for a in range(3):
                for b_ in range(3):
                    if axis == 0:
                        vals = Wm[ch, :, a, b_]
                    elif axis == 1:
                        vals = Wm[ch, a, :, b_]
                    else:
                        vals = Wm[ch, a, b_, :]
                    out[ch, a, b_] = band(vals * s, n)
        return out.reshape(18, n, n).astype(bf16)

    winC = np.zeros((128, 119), np.float32)
    winC[:, 55] = 1.0
    oneh = np.zeros((2, 128), np.float32)
    oneh[0, 127] = 1.0
    oneh[1, 0] = 1.0
    return {
        "oneh": oneh.astype(bf16),
        "bandWc": bands18(Wc[0], 1, 56, 1.0 / C),    # band over h; shifts (dt, dw)
        "bandWt": bands18(Wt[0], 0, 128, 1.0 / T),   # band over c(D); shifts (dh, dw)
        "bandWh": bands18(Wh[0], 1, 128, 1.0 / H),   # band over c(H); shifts (dt, dw)
        "bandWw": bands18(Ww[0], 2, 128, 1.0 / W),   # band over c(W); shifts (dt, dh)
        "winC": winC.astype(bf16),
    }


def build_program(Wc, Wt, Wh, Ww, gamma, beta):
    import concourse.bass as bass
    import concourse.tile as tile
    from concourse import mybir

    _patch_tile_drain()

    F32, BF16 = mybir.dt.float32, mybir.dt.bfloat16
    AF = mybir.ActivationFunctionType
    ALU = mybir.AluOpType
    bnsc = [float(gamma[i]) / float(np.sqrt(1.0 + EPS)) for i in range(4)]
    bnbi = [float(beta[i]) for i in range(4)]
    WtE = [np.asarray(Wt[0, :, 2, :, :]), np.asarray(Wt[0, :, 0, :, :])]
    WhE = [np.asarray(Wh[0, :, :, 2, :]), np.asarray(Wh[0, :, :, 0, :])]
    WwE = [np.asarray(Ww[0, :, :, :, 2]), np.asarray(Ww[0, :, :, :, 0])]
    MSC = {"t": 1.0 / T, "h": 1.0 / H, "w": 1.0 / W}

    nc = bass.Bass("TRN2", target_bir_lowering=False, debug=False)
    x_in = nc.declare_dram_parameter("x", [T, C, H, W], F32, isOutput=False)
    out_d = nc.declare_dram_parameter("out", [T, C, H, W], F32, isOutput=True)
    cWc = nc.declare_dram_parameter("bandWc", [18, 56, 56], BF16, isOutput=False)
    cWt = nc.declare_dram_parameter("bandWt", [18, 128, 128], BF16, isOutput=False)
    cWh = nc.declare_dram_parameter("bandWh", [18, 128, 128], BF16, isOutput=False)
    cWw = nc.declare_dram_parameter("bandWw", [18, 128, 128], BF16, isOutput=False)
    cwinC = nc.declare_dram_parameter("winC", [128, 119], BF16, isOutput=False)
    coneh = nc.declare_dram_parameter("oneh", [2, 128], BF16, isOutput=False)

    def bc(base, freedims, extra=0):
        return bass.AP(tensor=base.tensor, offset=base.offset + extra,
                       ap=[base.ap[0]] + freedims)

    import contextlib
    with tile.TileContext(nc) as tc, contextlib.ExitStack() as ctx:
        singles = ctx.enter_context(tc.tile_pool(name="singles", bufs=1))
        bigp = ctx.enter_context(tc.tile_pool(name="big", bufs=1))
        xtp = ctx.enter_context(tc.tile_pool(name="xt", bufs=1))
        ycbp = ctx.enter_context(tc.tile_pool(name="ycb", bufs=2))
        scr = ctx.enter_context(tc.tile_pool(name="scr", bufs=1))
        edgep = ctx.enter_context(tc.tile_pool(name="edgep", bufs=1))
        sm = ctx.enter_context(tc.tile_pool(name="sm", bufs=1))
        psp = ctx.enter_context(tc.tile_pool(name="ps", bufs=1, space="PSUM"))
        pscv = ctx.enter_context(tc.tile_pool(name="pscv", bufs=2, space="PSUM"))

        for _eng in (nc.vector, nc.scalar, nc.tensor, nc.gpsimd, nc.sync):
            _eng.nop(hint="wsplit_template")

        bWc = singles.tile([56, 18, 56], BF16, tag="bWc")
        nc.sync.dma_start(out=bWc[:], in_=cWc[:].rearrange("a b c -> b a c"))
        bWt = singles.tile([128, 18, 128], BF16, tag="bandcv", name="bWt")
        nc.sync.dma_start(out=bWt[:], in_=cWt[:].rearrange("a b c -> b a c"))
        bWh = singles.tile([128, 18, 128], BF16, tag="bandcv", name="bWh")
        nc.sync.dma_start(out=bWh[:], in_=cWh[:].rearrange("a b c -> b a c"))
        bWw = singles.tile([128, 18, 128], BF16, tag="bandcv", name="bWw")
        nc.sync.dma_start(out=bWw[:], in_=cWw[:].rearrange("a b c -> b a c"))
        winC = singles.tile([128, 119], BF16, tag="winC")
        nc.sync.dma_start(out=winC[:], in_=cwinC[:])
        oneh127 = singles.tile([1, 128], BF16, tag="oneh127")
        nc.sync.dma_start(out=oneh127[:], in_=coneh[0:1, :])
        oneh0 = singles.tile([1, 128], BF16, tag="oneh0")
        nc.sync.dma_start(out=oneh0[:], in_=coneh[1:2, :])

        x16 = [[bigp.tile([128, FLATP], BF16, tag=f"x16_{t}_{g}", name=f"x16_{t}_{g}")
                for g in range(2)] for t in range(T)]
        tpa = [[bigp.tile([128, 3364], BF16, tag=f"tp_{g}_{s}", name=f"tp_{g}_{s}")
                for s in range(2)] for g in range(2)]
        hpa = [[bigp.tile([128, 580], BF16, tag=f"hp_{g}_{s}", name=f"hp_{g}_{s}")
                for s in range(2)] for g in range(2)]
        wpa = [[bigp.tile([128, 580], BF16, tag=f"wp_{g}_{s}", name=f"wp_{g}_{s}")
                for s in range(2)] for g in range(2)]
        yh3 = [bigp.tile([128, T, W], BF16, tag=f"yh3_{g}", name=f"yh3_{g}") for g in range(2)]
        yw3 = [bigp.tile([128, T, H], BF16, tag=f"yw3_{g}", name=f"yw3_{g}") for g in range(2)]
        cpmax_T = sm.tile([128, T, 32], BF16, tag="cpmaxT")
        cpmax_TT = sm.tile([128, 2, 128], BF16, tag="cpmaxTT")
        cp_main = sm.tile([56, 2, 10, 58], BF16, tag="cpmain")
        cpsum_r = sm.tile([64, 448], BF16, tag="cpsumr")
        cpmax_f = sm.tile([8, FLATP], BF16, tag="cpmaxf")
        yc = sm.tile([56, T, W], BF16, tag="yc")
        yc_flat = sm.tile([8, FLAT], BF16, tag="cpmaxf", name="yc_flat")

        for t in range(T):
            for g in range(2):
                nc.vector.memset(bc(x16[t][g][:], [[1, FLATP - FLAT]], FLAT), 0.0)
        for g in range(2):
            for s in range(2):
                nc.vector.memset(tpa[g][s][:], 0.0)
                nc.vector.memset(hpa[g][s][:], 0.0)
                nc.vector.memset(wpa[g][s][:], 0.0)
        nc.vector.memset(cpmax_T[:], 0.0)
        nc.vector.memset(cp_main[:], 0.0)

        # ---- load (cast f32->bf16) ----
        for t in range(T):
            for g in range(2):
                nc.gpsimd.dma_start(
                    out=bc(x16[t][g][:], [[1, FLAT]]),
                    in_=x_in[t, g * 128:(g + 1) * 128, :, :].rearrange("c h w -> c (h w)"))

        # ---- transpose + c-max tree ----
        for t in range(T):
            xT = xtp.tile([128, NCH, 256], BF16, tag="xT")
            for g in range(2):
                nc.sync.dma_start(
                    out=bc(xT[:], [[256, NCH], [1, 128]], g * 128),
                    in_=x16[t][g][:], transpose=True)
            cm = scr.tile([128, NCH, 128], BF16, tag="scratch", name="cm")
            nc.vector.tensor_tensor(out=cm[:], in0=xT[:, :, 0:128], in1=xT[:, :, 128:256], op=ALU.max)
            w_ = 64
            while w_ >= 2:
                nc.vector.tensor_tensor(out=cm[:, :, 0:w_], in0=cm[:, :, 0:w_],
                                        in1=cm[:, :, w_:2 * w_], op=ALU.max)
                w_ //= 2
            nc.vector.tensor_tensor(
                out=cpmax_T[:, t, 0:NCH],
                in0=bc(cm[:], [[128, NCH], [1, 1]], 0),
                in1=bc(cm[:], [[128, NCH], [1, 1]], 1), op=ALU.max)

        # ---- c-sum staircase (PE) ----
        ps_cs = psp.tile([64, 448], F32, tag="ps_cs")
        for t in range(T):
            for o in range(7):
                r = t * 7 + o
                for g in range(2):
                    nc.tensor.matmul(
                        ps_cs[:], winC[:, 55 - r:119 - r],
                        bc(x16[t][g][:], [[1, 448]], o * 448),
                        start=(r == 0 and g == 0), stop=(r == 55 and g == 1))
        nc.vector.tensor_copy(out=cpsum_r[:], in_=ps_cs[:])

        # ---- assemble cp_main ----
        for t in range(T):
            nc.sync.dma_start(
                out=bc(cp_main[:], [[1, 56]], 580 + (t + 1) * 58 + 1),
                in_=cpsum_r[t * 7:(t + 1) * 7, :])
        nc.sync.dma_start(out=cpmax_TT[:], in_=cpmax_T[:].rearrange("p a b -> p (a b)"),
                          transpose=True)
        for t in range(T):
            blk, row0 = divmod(t * 32, 128)
            nc.sync.dma_start(out=cpmax_f[t:t + 1, :], in_=cpmax_TT[row0:row0 + NCH, blk, :])
            nc.sync.dma_start(
                out=bc(cp_main[:], [[1, 56]], 0 + (t + 1) * 58 + 1),
                in_=cpmax_f[t:t + 1, 0:FLAT])

        # ---- Wc conv + sigmoid -> yc ----
        ps_yc = psp.tile([56, 448], F32, tag="ps_yc")
        k = 0
        for ch in range(2):
            for dt in range(3):
                for dw in range(3):
                    nc.tensor.matmul(
                        ps_yc[:], bWc[:, ch * 9 + dt * 3 + dw, :],
                        bc(cp_main[:], [[58, 8], [1, 56]], ch * 580 + dt * 58 + dw),
                        start=(k == 0), stop=(k == 17))
                    k += 1
        nc.scalar.activation(out=yc[:].rearrange("h t w -> h (t w)"), in_=ps_yc[:],
                             func=AF.Sigmoid, bias=bnbi[0], scale=bnsc[0])
        for t in range(T):
            nc.sync.dma_start(out=yc_flat[t:t + 1, :], in_=yc[:, t, :])

        # ---- xc = x * yc (in place) ----
        import concourse.bass as _b
        for t in range(T):
            ycb = ycbp.tile([128, FLAT], BF16, tag="ycb")
            nc.sync.dma_start(
                out=ycb[:],
                in_=_b.AP(tensor=yc_flat[:].tensor, offset=yc_flat[:].offset + t * FLAT,
                          ap=[[FLAT, 1], [0, 128], [1, FLAT]]))
            for g in range(2):
                xs = bc(x16[t][g][:], [[1, FLAT]])
                nc.vector.tensor_tensor(out=xs, in0=xs, in1=ycb[:], op=ALU.mult)

        # ---- t/h/w pools ----
        for g in range(2):
            for s, op in ((0, ALU.max), (1, ALU.add)):
                dst = bc(tpa[g][s][:], [[58, 56], [1, 56]], 59)
                nc.vector.tensor_tensor(out=dst, in0=bc(x16[0][g][:], [[1, FLAT]]),
                                        in1=bc(x16[1][g][:], [[1, FLAT]]), op=op)
                for t in range(2, T):
                    nc.vector.tensor_tensor(out=dst, in0=dst,
                                            in1=bc(x16[t][g][:], [[1, FLAT]]), op=op)
        for t in range(T):
            for g in range(2):
                xg = x16[t][g][:]
                for s, op in ((0, ALU.max), (1, ALU.add)):
                    hs = scr.tile([128, 28, 56], BF16, tag="scratch", name="hs")
                    nc.vector.tensor_tensor(out=hs[:], in0=bc(xg, [[56, 28], [1, 56]], 0),
                                            in1=bc(xg, [[56, 28], [1, 56]], 28 * 56), op=op)
                    n = 28
                    while n > 1:
                        h_ = n // 2
                        nc.vector.tensor_tensor(out=hs[:, 0:h_, :], in0=hs[:, 0:h_, :],
                                                in1=hs[:, h_:2 * h_, :], op=op)
                        if n % 2:
                            nc.vector.tensor_tensor(out=hs[:, 0:1, :], in0=hs[:, 0:1, :],
                                                    in1=hs[:, n - 1:n, :], op=op)
                        n = h_
                    nc.vector.tensor_copy(
                        out=bc(hpa[g][s][:], [[1, 56]], (t + 1) * 58 + 1), in_=hs[:, 0, :])
                    ws = scr.tile([128, 56, 28], BF16, tag="scratch", name="ws")
                    nc.vector.tensor_tensor(out=ws[:], in0=bc(xg, [[56, 56], [1, 28]], 0),
                                            in1=bc(xg, [[56, 56], [1, 28]], 28), op=op)
                    n = 28
                    while n > 1:
                        h_ = n // 2
                        nc.vector.tensor_tensor(out=ws[:, :, 0:h_], in0=ws[:, :, 0:h_],
                                                in1=ws[:, :, h_:2 * h_], op=op)
                        if n % 2:
                            nc.vector.tensor_tensor(out=ws[:, :, 0:1], in0=ws[:, :, 0:1],
                                                    in1=ws[:, :, n - 1:n], op=op)
                        n = h_
                    nc.vector.tensor_copy(
                        out=bc(wpa[g][s][:], [[1, 56]], (t + 1) * 58 + 1), in_=ws[:, :, 0])

        # ---- gates: yt3 reuses x16_0 slots ----
        yt3 = [bigp.tile([128, FLAT], BF16, tag=f"x16_0_{g}", name=f"yt3_{g}") for g in range(2)]

        cfT = [edgep.tile([1, FLAT], BF16, tag=f"cfT{d}", name=f"cfT{d}") for d in range(2)]
        cfH = [edgep.tile([1, 448], BF16, tag=f"cfH{d}", name=f"cfH{d}") for d in range(2)]
        cfW = [edgep.tile([1, 448], BF16, tag=f"cfW{d}", name=f"cfW{d}") for d in range(2)]

        def make_corr(pools, wE, kind, cfs):
            nr = 56 if kind == "t" else 8
            for d in range(2):
                src_g, src_p = (1, 0) if d == 0 else (0, 127)
                e3 = edgep.tile([58, 3, 2, 60], BF16, tag="edge", name=f"edge_{kind}_{d}")
                nc.vector.memset(e3[:], 0.0)
                for s_ in range(2):
                    for a, (dst0, cnt, srcoff) in enumerate((
                            (1, nr, 0),         # e3[p] = field[p-1]
                            (0, nr, 0),         # e3[p] = field[p]
                            (0, nr - 1, 58))):  # e3[p] = field[p+1]
                        nc.sync.dma_start(
                            out=_b.AP(tensor=e3[:].tensor,
                                      offset=e3[:].offset + dst0 * 360 + a * 120 + s_ * 60 + 1,
                                      ap=[[360, cnt], [1, 56]]),
                            in_=bc(pools[src_g][s_][src_p:src_p + 1, :],
                                   [[58, cnt], [1, 56]], 59 + srcoff))
                corr = edgep.tile([58, 56], BF16, tag="corr", name=f"corr_{kind}_{d}")
                nc.vector.memset(corr[:], 0.0)
                wm = wE[d]
                for ch in range(2):
                    sc = MSC[kind] if ch == 1 else 1.0
                    for a in range(3):
                        for b_ in range(3):
                            wv = float(wm[ch, a, b_]) * sc
                            nc.vector.scalar_tensor_tensor(
                                out=corr[0:nr, 0:56],
                                in0=_b.AP(tensor=e3[:].tensor,
                                          offset=e3[:].offset + a * 120 + ch * 60 + b_,
                                          ap=[[360, nr], [1, 56]]),
                                scalar=wv, in1=corr[0:nr, 0:56],
                                op0=ALU.mult, op1=ALU.add)
                nc.sync.dma_start(out=cfs[d][:, 0:nr * 56], in_=corr[0:nr, 0:56])

        make_corr(tpa, WtE, "t", cfT)
        make_corr(hpa, WhE, "h", cfH)
        make_corr(wpa, WwE, "w", cfW)

        # ---- Wt conv (banded over c) + sigmoid -> yt3 ----
        for g in range(2):
            for o in range(7):
                ps = pscv.tile([128, 448], F32, tag="ps_cv", name="ps_wt")
                k = 0
                for ch in range(2):
                    for dh in range(3):
                        for dw in range(3):
                            nc.tensor.matmul(
                                ps[:], bWt[:, ch * 9 + dh * 3 + dw, :],
                                bc(tpa[g][ch][:], [[58, 8], [1, 56]], (8 * o + dh) * 58 + dw),
                                start=(k == 0), stop=False)
                            k += 1
                nc.tensor.matmul(ps[:], oneh127 if g == 0 else oneh0,
                                 cfT[g][:, o * 448:(o + 1) * 448], start=False, stop=True)
                sg = scr.tile([128, 448], BF16, tag="scratch", name="sg")
                nc.scalar.activation(out=sg[:], in_=ps[:], func=AF.Sigmoid,
                                     bias=bnbi[1], scale=bnsc[1])
                nc.vector.tensor_scalar(out=yt3[g][:, o * 448:(o + 1) * 448], in0=sg[:],
                                        scalar1=1.0 / 3.0, scalar2=None, op0=ALU.mult)

        # ---- Wh conv -> yh3 ; Ww conv -> yw3 ----
        for which, bmat, pools, cfs, ydst, bi in (
                ("h", bWh, hpa, cfH, yh3, 2), ("w", bWw, wpa, cfW, yw3, 3)):
            for g in range(2):
                ps = pscv.tile([128, 448], F32, tag="ps_cv", name=f"ps_{which}")
                k = 0
                for ch in range(2):
                    for dt in range(3):
                        for db in range(3):
                            nc.tensor.matmul(
                                ps[:], bmat[:, ch * 9 + dt * 3 + db, :],
                                bc(pools[g][ch][:], [[58, 8], [1, 56]], dt * 58 + db),
                                start=(k == 0), stop=False)
                            k += 1
                nc.tensor.matmul(ps[:], oneh127 if g == 0 else oneh0,
                                 cfs[g][:], start=False, stop=True)
                sg = scr.tile([128, 448], BF16, tag="scratch", name="sg")
                nc.scalar.activation(out=sg[:], in_=ps[:], func=AF.Sigmoid,
                                     bias=bnbi[bi], scale=bnsc[bi])
                nc.vector.tensor_scalar(out=ydst[g][:].rearrange("p a b -> p (a b)"),
                                        in0=sg[:], scalar1=1.0 / 3.0, scalar2=None, op0=ALU.mult)

        # ---- final: out = yt3 + yh3(bcast h) + yw3(bcast w), cast-store ----
        for t in range(T):
            for g in range(2):
                v = bigp.tile([128, FLAT], BF16, tag=f"x16_{max(t, 1)}_{g}", name=f"v_{t}_{g}")
                nc.vector.tensor_tensor(
                    out=v[:], in0=yt3[g][:],
                    in1=bc(yh3[g][:], [[0, 56], [1, 56]], t * 56), op=ALU.add)
                nc.vector.tensor_tensor(
                    out=v[:], in0=v[:],
                    in1=bc(yw3[g][:], [[1, 56], [0, 56]], t * 56), op=ALU.add)
                nc.gpsimd.dma_start(
                    out=out_d[t, g * 128:(g + 1) * 128, :, :].rearrange("c h w -> c (h w)"),
                    in_=v[:])
    _split_multiwaits(nc, mybir)
    return nc


# Per-instruction sync-wait slot capacity of this walrus build (discovered
# empirically; excess waits are moved onto inserted same-engine nops).
WAIT_CAPS = {}
DEFAULT_WAIT_CAP = 1


def _split_multiwaits(nc, mybir):
    import copy
    templates = {}
    blocks = nc.m.functions[0].blocks
    for bb in blocks:
        for inst in bb.instructions:
            if type(inst).__name__ == "InstNoOp" and inst.engine not in templates:
                templates[inst.engine] = inst
    ctr = 0
    for bb in blocks:
        newl = []
        for inst in bb.instructions:
            si = getattr(inst, "sync_info", None)
            if si is not None and si.on_wait:
                cap = WAIT_CAPS.get(type(inst).__name__, DEFAULT_WAIT_CAP)
                waits = list(si.on_wait)
                if len(waits) > cap:
                    keep = waits[-cap:]
                    extra = waits[:-cap]
                    si.on_wait = keep
                    tpl = templates.get(inst.engine)
                    assert tpl is not None, f"no nop template for {inst.engine}"
                    nop_cap = WAIT_CAPS.get("InstNoOp", 1)
                    for i in range(0, len(extra), nop_cap):
                        nop = copy.deepcopy(tpl)
                        ctr += 1
                        nop.name = f"WSPLIT-{ctr}"
                        nop.sync_info = mybir.SyncInfo(
                            on_wait=extra[i:i + nop_cap], on_update=[])
                        newl.append(nop)
            newl.append(inst)
        if len(newl) != len(bb.instructions):
            bb.instructions[:] = newl
    return ctr


_CACHE = {}


def _get_program(Wc, Wt, Wh, Ww, gamma, beta):
    key = hash((Wc.tobytes(), Wt.tobytes(), Wh.tobytes(), Ww.tobytes(),
                gamma.tobytes(), beta.tobytes()))
    if key not in _CACHE:
        _CACHE[key] = (build_program(Wc, Wt, Wh, Ww, gamma, beta),
                       make_consts(Wc, Wt, Wh, Ww))
    return _CACHE[key]


def kernel(**inputs):
    x = np.ascontiguousarray(np.asarray(inputs["x"], dtype=np.float32))
    Wc = np.asarray(inputs["Wc"], dtype=np.float32)
    Wt = np.asarray(inputs["Wt"], dtype=np.float32)
    Wh = np.asarray(inputs["Wh"], dtype=np.float32)
    Ww = np.asarray(inputs["Ww"], dtype=np.float32)
    gamma = np.asarray(inputs["bn_gamma"], dtype=np.float32)
    beta = np.asarray(inputs["bn_beta"], dtype=np.float32)

    nc, consts = _get_program(Wc, Wt, Wh, Ww, gamma, beta)

    from concourse.bass_utils import run_bass_kernel_spmd

    in_maps = []
    for b in range(NCORES):
        m = {"x": x[b * T:(b + 1) * T].reshape(T, C, H, W)}
        m.update(consts)
        in_maps.append(m)
    res = run_bass_kernel_spmd(nc, in_maps, core_ids=list(range(NCORES)))
    out = np.empty((NCORES * T, C, H, W), np.float32)
    for b in range(NCORES):
        out[b * T:(b + 1) * T] = res.results[b]["out"]
    return out

